# revision 40
# baseline (speedup 1.0000x reference)
"""Binary conv2d (XNOR-style) + per-channel scale for Trainium2.

y = conv2d(sign(x), sign(w), stride=1, pad=1) * scale[oc]

Data-parallel over batch across 8 NeuronCores (4 images each).

Per core, each image is binarized into a slot-interleaved padded fp8
layout [p=128, row(58), slot(2), 57]: the two in-channel blocks (slots)
of a padded row sit in adjacent 57-col segments, so a DoubleRow matmul
rhs AP [p, 2(slot @57), 8(row @114), 57(col @1)] has a row-local byte
footprint.  That makes the Tile framework's range-based dependencies
row-granular: image 0 streams in 8-row groups (DMA -> sign -> matmul)
instead of waiting for the whole image, and the 57-stride pad-sharing
trick still works because col 0 of every segment is a zeroed left pad.

The 3x3 conv is 9 accumulating DoubleRow matmuls (K=256) per 8-row
output chunk into a PSUM bank; per-channel scale is applied in fp32 on
the PSUM->SBUF drain, so the result is bit-identical to the reference.
Weights are pre-binarized and packed to fp8 on the host (they are
static parameters), quartering their DMA and removing on-chip signs.

Schedule notes:
- ~80 tiny dummy matmuls at t=0 ramp the PE DVFS p-state while the
  first DMAs land, so real matmuls start at full clock.
- PSUM chunk groups of 2-4 banks (7 banks in the pool + 1 warmup) keep
  bank recycling ahead of the drains.
- Output chunks drain (with scale) into per-(img,oc-block) SBUF tiles,
  stored as half-images (small per-group stores for the last image to
  shorten the kernel tail); stores ride the SP ring, image loads the
  ACT/DVE rings.
"""

import numpy as np

N_CORES = 8
IMGS = 4  # images per core
IC = 256
OC = 256
H = W = 56
SEG = 57  # 57-col segment: col0 = left pad (shared as prev seg's right pad)
RSTR = 2 * SEG  # padded row stride: slot0 seg + slot1 seg
NPROWS = 58  # 56 data rows + top/bottom pad
IMG_DATA = NPROWS * RSTR  # 6612
IMG_F = 6624  # padded to mult of 16; tail slack is zeroed (window overrun)
ROWS = 8  # output rows per PSUM tile
NFREE = ROWS * SEG  # 456 <= 512 (PSUM bank limit)
NCHUNK = H // ROWS  # 7
G_ROWS = 8  # image-0 streaming row-group size
N_G = H // G_ROWS  # 7
STEADY_SPLIT = 36  # steady images: ACT signs rows [0,36), DVE [36,56)

_cache = {}

# Skip the Tile kernel-tail drain + semaphore-clear protocol entirely.  The
# trace shows it costs ~8us after the last store (a per-engine semaphore
# handshake storm).  Safe iff the runtime hands each execution zeroed
# semaphores (verified empirically: repeated executions stay bit-exact).
SKIP_TAIL = True


def _install_drain_patch():
    """This walrus build rejects >1 sync-wait on ctrl-type instructions;
    Tile's kernel-tail drain carries one wait per pending proc.  Split it
    into one drain per proc (each with <=1 wait)."""
    import concourse.tile as _tile
    from concourse.vector_clock import ScopedClock, VectorClock

    if getattr(_tile.TileContext, "_drain_split_patch", False):
        return

    def _drain_and_barrier(self, tick_clock, wait_clock):
        nc = self.nc
        assert self.sems is not None
        popped = nc._tile_sem_poison_stack.pop()
        assert popped is self._sem_poison
        if SKIP_TAIL:
            # No drains, no sem clear: the engine streams simply end.  NRT
            # completion waits for all DMA queues regardless, and each
            # execution starts with freshly-zeroed semaphores.
            return
        # Drains AND the sem clears both run on the Pool engine (the
        # framework's clear_and_free_semaphores emits gpsimd dma_reset/
        # sem_clear), so the first all-engine barrier of the stock tail is
        # unnecessary: once Pool's drains observe every proc's final tick,
        # all sem increments have retired and Pool may clear immediately.
        gclock = tick_clock.global_clock
        n = len(gclock)
        for p in range(n):
            t = gclock[p]
            if t <= 0:
                continue
            vec = [0] * n
            vec[p] = t
            d = nc.gpsimd.drain()
            wait_clock.add_sem_waits(d.ins, ScopedClock({None: VectorClock(vec)}))
        nc.clear_and_free_semaphores(list(self.sems.allocated().values()))
        # No final all-engine barrier: every other engine's stream simply
        # ends, NRT completion waits for all queues anyway, and the next
        # invocation cannot start before this one fully retires.

    _tile.TileContext._drain_and_barrier = _drain_and_barrier
    _tile.TileContext._drain_split_patch = True


def _split_excess_waits(nc, maxw=1):
    """Same walrus limitation: hoist excess sync-waits onto same-engine
    NoOps inserted just before the instruction (engine streams are
    in-order, so a preceding NoOp carrying the waits is equivalent)."""
    import concourse.mybir as mybir

    n_split = 0
    for f in nc.m.functions:
        for bb in f.blocks:
            out = []
            for ins in bb.instructions:
                si = ins.sync_info
                if si and si.on_wait and len(si.on_wait) > maxw:
                    waits = list(si.on_wait)
                    excess, keep = waits[:-maxw], waits[-maxw:]
                    for i in range(0, len(excess), maxw):
                        nop = mybir.InstNoOp(
                            name=f"{ins.name}_waitsplit{i}",
                            engine=ins.engine,
                            ins=[],
                            outs=[],
                            sync_info=mybir.SyncInfo(
                                on_wait=excess[i : i + maxw], on_update=[]
                            ),
                        )
                        out.append(nop)
                    si.on_wait = keep
                    n_split += 1
                out.append(ins)
            bb.instructions = out
    return n_split


def build_nc():
    import concourse.bass as bass
    import concourse.mybir as mybir
    from concourse.tile import TileContext

    _install_drain_patch()

    # The Bass-preamble all-engine barrier re-runs at every block-dispatch
    # loop-around; its per-engine DMA-queue drains put a serial ~4us
    # drain+barrier chain after the last store (NRT already gates NEFF
    # completion on DMA-queue idle, so the drains are redundant).  Use the
    # sem-only barrier variant instead.
    if not getattr(bass.Bass, "_sem_only_aeb_patch", False):
        _orig_aeb = bass.Bass.all_engine_barrier

        def _aeb(self, *, sem_only=False):
            return _orig_aeb(self, sem_only=True)

        bass.Bass.all_engine_barrier = _aeb
        bass.Bass._sem_only_aeb_patch = True

    f32 = mybir.dt.float32
    bdt = mybir.dt.float8e4
    Copy = mybir.ActivationFunctionType.Copy
    DR = mybir.MatmulPerfMode.DoubleRow

    nc = bass.Bass()
    x = nc.declare_dram_parameter("x", [IMGS, IC, H, W], f32, isOutput=False)
    # host-binarized weights: [p=128(ic in block), 9 taps x 2 icb, 256 oc] fp8
    wtb = nc.declare_dram_parameter("wtb", [128, 18, OC], bdt, isOutput=False)
    scale = nc.declare_dram_parameter("scale", [OC], f32, isOutput=False)
    y = nc.declare_dram_parameter("y", [IMGS, OC, H, W], f32, isOutput=True)

    with TileContext(nc) as tc:
        with (
            tc.tile_pool(name="const", bufs=1) as cpool,
            tc.tile_pool(name="g0", bufs=7) as g_pool,
            tc.tile_pool(name="tmp", bufs=4) as tmp_pool,
            tc.tile_pool(name="xin", bufs=2) as xin_pool,
            tc.tile_pool(name="outp", bufs=4) as out_pool,
            tc.tile_pool(name="psum", bufs=7, space="PSUM") as psum_pool,
            tc.tile_pool(name="warm", bufs=1, space="PSUM") as warm_pool,
        ):
            # --- PE p-state warmup: tiny self-contained matmuls with no
            # data deps keep the PE array busy (ramping DVFS) while the
            # first image DMAs land; sized to finish just before the first
            # real matmul's inputs are ready.  Inputs are a zeroed scratch
            # tile; the PSUM bank is dedicated and never read.
            warmw = cpool.tile([128, 2, 128], bdt)
            nc.vector.memset(warmw[:], 0.0)
            warmp = warm_pool.tile([128, 64], f32)
            for _ in range(46):
                nc.tensor.matmul(
                    warmp[:], warmw[:], warmw[:, :, 0:64],
                    start=True, stop=True, perf_mode=DR, skip_group_check=True,
                )

            # --- weights (pre-signed fp8 from host) on the ACT ring, which
            # carries nothing else at startup: taps 0-1 first (they gate the
            # first matmuls), then the rest.
            wb = cpool.tile([128, 18, OC], bdt)
            nc.scalar.dma_start(out=wb[:, 0:4, :], in_=wtb[:, 0:4, :])
            nc.scalar.dma_start(out=wb[:, 4:18, :], in_=wtb[:, 4:18, :])

            # --- padding rings: data-independent, zeroed on the GpSimd
            # engine.  Image 0's pads first (they gate its signs), the rest
            # after image 0's streaming loads are on the SWDGE ring.
            xp = cpool.tile([128, IMGS, IMG_F], bdt)

            def pad_image(n):
                v = xp[:, n, 0:IMG_DATA].rearrange("p (r s c) -> p r s c", s=2, c=SEG)
                nc.gpsimd.memset(xp[:, n, 0:RSTR], 0.0)  # top pad row
                nc.gpsimd.memset(xp[:, n, (NPROWS - 1) * RSTR : IMG_F], 0.0)  # bottom+tail
                nc.gpsimd.memset(v[:, 1 : NPROWS - 1, :, 0:1], 0.0)  # left pads

            pad_image(0)

            # scale first on the SP queue (tiny; the first PSUM drains wait
            # on it, so it must not sit behind image loads)
            sc = cpool.tile([128, 2], f32)
            nc.sync.dma_start(out=sc[:], in_=scale.rearrange("(b p) -> p b", p=128))

            def img_view(n):
                return xp[:, n, 0:IMG_DATA].rearrange(
                    "p (r s c) -> p r s c", s=2, c=SEG
                )

            def sign_dst(n, icb, r0, r1):
                # x rows [r0, r1) of slot icb -> padded rows [r0+1, r1+1),
                # data cols 1..56 of that slot's segment
                return img_view(n)[:, r0 + 1 : r1 + 1, icb, 1:57]

            def act_sign(n, icb, xin_slice, r0, r1):
                nc.scalar.sign(sign_dst(n, icb, r0, r1), xin_slice)

            def dve_sign(n, icb, xin_slice, r0, r1, tag="tmp"):
                # sign() on the vector engine: clamp(x * 1e38, -1, 1).
                # Exact for fp32 normals and +-0; frees ScalarE.
                tmp = tmp_pool.tile(
                    [128, r1 - r0, W], f32, name=f"tmp{n}_{icb}_{r0}", tag=tag
                )
                nc.vector.tensor_scalar(
                    tmp[:], xin_slice, 1.0e38, -1.0,
                    op0=mybir.AluOpType.mult, op1=mybir.AluOpType.max,
                )
                nc.vector.tensor_scalar_min(sign_dst(n, icb, r0, r1), tmp[:], 1.0)

            # Both ic blocks of an image (or row range) load in ONE DMA:
            # the DRAM AP folds channel = icb*128 + p into [p, icb, h, w].
            xpair = x.rearrange("n (b p) h w -> n p b h w", p=128)

            def load_pair(n, r0, r1, eng, name):
                t = g_pool.tile(
                    [128, 2, r1 - r0, W], f32, name=name, tag="g0in"
                ) if r1 - r0 <= 8 else xin_pool.tile(
                    [128, 2, H, W], f32, name=name, tag="xin"
                )
                if r1 - r0 <= 8:
                    eng.dma_start(out=t[:], in_=xpair[n, :, :, r0:r1, :])
                else:
                    # steady image: two half-DMAs on different queues so
                    # both halves stream in parallel; the split matches the
                    # ACT/DVE sign split so each engine's sign starts as
                    # soon as its half lands.
                    s = STEADY_SPLIT
                    nc.sync.dma_start(
                        out=t[:, :, 0:s, :], in_=xpair[n, :, :, 0:s, :]
                    )
                    nc.gpsimd.dma_start(
                        out=t[:, :, s:H, :], in_=xpair[n, :, :, s:H, :]
                    )
                return t

            # --- image 0: stream 8-row groups, one pair-DMA each,
            # alternating the SP and GpSimd-SWDGE queues (DMA issues
            # serialize at ~0.6us on a sequencer).  Signs: even groups on
            # ACT (one op per slot), odd on DVE; group boxes stay
            # engine-disjoint.  Chunk c's matmuls unlock once rows <=
            # 8c+9 are signed (range-based deps).
            g_tiles = []
            for g in range(N_G):
                r0, r1 = g * G_ROWS, (g + 1) * G_ROWS
                eng = nc.sync if g % 2 == 0 else nc.gpsimd
                g_tiles.append(load_pair(0, r0, r1, eng, f"g{g}"))
            for g in range(N_G):
                r0, r1 = g * G_ROWS, (g + 1) * G_ROWS
                t = g_tiles[g]
                if g % 2 == 0:
                    act_sign(0, 0, t[:, 0, :, :], r0, r1)
                    act_sign(0, 1, t[:, 1, :, :], r0, r1)
                else:
                    dve_sign(0, 0, t[:, 0, :, :], r0, r1)
                    dve_sign(0, 1, t[:, 1, :, :], r0, r1)

            # remaining pads behind image 0's streaming loads
            for n in range(1, IMGS):
                pad_image(n)

            def load_image(n, eng):
                return load_pair(n, 0, H, eng, f"xin{n}")

            def sign_image(n, xin):
                # ACT signs rows [0, SPLIT) of both slots, DVE the rest:
                # row-disjoint boxes so the engines run in parallel.
                s = STEADY_SPLIT
                act_sign(n, 0, xin[:, 0, 0:s, :], 0, s)
                act_sign(n, 1, xin[:, 1, 0:s, :], 0, s)
                dve_sign(n, 0, xin[:, 0, s:H, :], s, H)
                dve_sign(n, 1, xin[:, 1, s:H, :], s, H)

            def store(n, ocb, c0, c1, eng):
                eng.dma_start(
                    out=y[n, ocb * 128 : (ocb + 1) * 128, c0 * ROWS : c1 * ROWS, :],
                    in_=_obufs[(n, ocb)][:, c0 * ROWS * W : c1 * ROWS * W].rearrange(
                        "p (h w) -> p h w", w=W
                    ),
                )

            _obufs = {}

            def compute_image(n, subs, small_stores=False, mid=None):
                # tap-outer (weight-stationary) within each chunk subgroup so
                # consecutive matmuls hit different PSUM banks; subgroups of
                # 2-4 banks (pool holds 7) keep allocation ahead of drains.
                # `mid` (next image's signs) is emitted after the first
                # subgroup so it lands mid-stream on ACT/DVE: after this
                # image's early drains, before its late ones.
                for ocb in range(2):
                    _obufs[(n, ocb)] = out_pool.tile(
                        [128, H * W], f32, name=f"ob{n}{ocb}", tag="ob"
                    )
                for sub_i, (c0, c1) in enumerate(subs):
                    if sub_i == 1 and mid is not None:
                        mid()
                    for ocb in range(2):
                        psums = [
                            psum_pool.tile(
                                [128, NFREE], f32, name=f"ps{n}{ocb}{c}", tag="ps"
                            )
                            for c in range(c0, c1)
                        ]
                        for t in range(9):
                            kh, kw = divmod(t, 3)
                            lhsT = wb[:, 2 * t : 2 * t + 2, ocb * 128 : (ocb + 1) * 128]
                            for c in range(c0, c1):
                                off = (ROWS * c + kh) * RSTR + kw
                                rhs = xp[:, n, off : off + ROWS * RSTR].rearrange(
                                    "p (r s c) -> p s r c", s=2, c=SEG
                                )
                                nc.tensor.matmul(
                                    psums[c - c0][:],
                                    lhsT,
                                    rhs,
                                    start=(t == 0),
                                    stop=(t == 8),
                                    perf_mode=DR,
                                )
                        for c in range(c0, c1):
                            src = psums[c - c0].rearrange("p (h w) -> p h w", w=SEG)[
                                :, :, 0:W
                            ]
                            dst = _obufs[(n, ocb)][
                                :, c * ROWS * W : (c + 1) * ROWS * W
                            ].rearrange("p (h w) -> p h w", w=W)
                            # drains: DVE takes 5/7 chunks, ACT 2/7 (ACT also
                            # carries weight/store DMA issues); chunks 5 and
                            # 6 are on different engines so the final image's
                            # last two drains run in parallel
                            if c in (1, 5):
                                nc.scalar.activation(
                                    dst, src, Copy, scale=sc[:, ocb : ocb + 1]
                                )
                            else:
                                nc.vector.tensor_scalar_mul(
                                    dst, src, sc[:, ocb : ocb + 1]
                                )
                            if small_stores:
                                # last image: store every chunk as soon as
                                # it drains — the kernel-end barrier waits
                                # for the slowest DMA queue, so spread the
                                # early chunks and put the final ones on
                                # the queues with nothing else pending.
                                if c < 4:
                                    eng = (nc.sync, nc.scalar, nc.gpsimd)[
                                        (c * 2 + ocb) % 3
                                    ]
                                else:
                                    eng = (nc.gpsimd, nc.scalar)[ocb]
                                store(n, ocb, c, c + 1, eng)
                if not small_stores:
                    # halves: oc-block 0 on the (store-only) ACT queue,
                    # oc-block 1 split over SP/GpSimd against their loads
                    store(n, 0, 0, 4, nc.scalar)
                    store(n, 0, 4, NCHUNK, nc.scalar)
                    store(n, 1, 0, 4, nc.gpsimd)
                    store(n, 1, 4, NCHUNK, nc.sync)

            # interleave: image n+1's loads are emitted (and thus queue-
            # prioritized) ahead of image n's compute; its signs are emitted
            # MID-compute so they slot between image n's early and late
            # drains in the ACT/DVE streams (a sign emitted before the
            # drains would block them in-order while waiting for its load).
            xin1 = load_image(1, nc.sync)
            compute_image(
                0, subs=((0, 2), (2, 4), (4, NCHUNK)),
                mid=lambda: sign_image(1, xin1),
            )
            xin2 = load_image(2, nc.gpsimd)
            compute_image(
                1, subs=((0, 4), (4, NCHUNK)),
                mid=lambda: sign_image(2, xin2),
            )
            xin3 = load_image(3, nc.sync)
            compute_image(
                2, subs=((0, 4), (4, NCHUNK)),
                mid=lambda: sign_image(3, xin3),
            )
            compute_image(
                3, subs=((0, 3), (3, 5), (5, 6), (6, NCHUNK)), small_stores=True
            )

    _split_excess_waits(nc)
    return nc


def _get_nc():
    if "nc" not in _cache:
        _cache["nc"] = build_nc()
    return _cache["nc"]


def _pack_weights(weight):
    """Host-side: binarize + lay out conv weights as [p, (kh kw icb), oc] fp8.

    Weight row k of the DoubleRow contraction is icb*128 + p, matching the
    rhs slot order (slot = icb)."""
    import concourse.mybir as mybir

    w = np.sign(weight.astype(np.float32))  # [oc, ic, kh, kw]
    w = w.transpose(2, 3, 1, 0).reshape(3 * 3, 2, 128, OC)  # [t, icb, p, oc]
    w = w.transpose(2, 0, 1, 3).reshape(128, 18, OC)  # [p, (t icb), oc]
    return np.ascontiguousarray(w).astype(mybir.dt.np(mybir.dt.float8e4))


def run(inputs, trace=False, trace_cores=None):
    from concourse.bass_utils import run_bass_kernel_spmd

    x = np.asarray(inputs["x"])
    weight = np.asarray(inputs["weight"])
    scale = np.asarray(inputs["scale"]).astype(np.float32)
    wtb = _pack_weights(weight)

    in_maps = [
        {"x": x[i * IMGS : (i + 1) * IMGS], "wtb": wtb, "scale": scale}
        for i in range(N_CORES)
    ]
    res = run_bass_kernel_spmd(
        _get_nc(),
        in_maps,
        core_ids=list(range(N_CORES)),
        trace=trace,
        trace_cores=trace_cores,
    )
    out = np.concatenate([res.results[i]["y"] for i in range(N_CORES)], axis=0)
    return out, res


def kernel(**inputs):
    # One retry: a previously crashed process can leave a core wedged
    # (NRT_EXEC_UNIT_UNRECOVERABLE); the runtime recovers on the next
    # attempt.
    try:
        out, _ = run(inputs, trace=False)
    except Exception:
        out, _ = run(inputs, trace=False)
    return out


# revision 41
# speedup vs baseline: 1.0256x; 1.0256x over previous
"""Binary conv2d (XNOR-style) + per-channel scale for Trainium2.

y = conv2d(sign(x), sign(w), stride=1, pad=1) * scale[oc]

Data-parallel over batch across 8 NeuronCores (4 images each).

Per core, each image is binarized into a slot-interleaved padded fp8
layout [p=128, row(58), slot(2), 57]: the two in-channel blocks (slots)
of a padded row sit in adjacent 57-col segments, so a DoubleRow matmul
rhs AP [p, 2(slot @57), 8(row @114), 57(col @1)] has a row-local byte
footprint.  That makes the Tile framework's range-based dependencies
row-granular: image 0 streams in 8-row groups (DMA -> sign -> matmul)
instead of waiting for the whole image, and the 57-stride pad-sharing
trick still works because col 0 of every segment is a zeroed left pad.

The 3x3 conv is 9 accumulating DoubleRow matmuls (K=256) per 8-row
output chunk into a PSUM bank; per-channel scale is applied in fp32 on
the PSUM->SBUF drain, so the result is bit-identical to the reference.
Weights are pre-binarized and packed to fp8 on the host (they are
static parameters), quartering their DMA and removing on-chip signs.

Schedule notes:
- ~80 tiny dummy matmuls at t=0 ramp the PE DVFS p-state while the
  first DMAs land, so real matmuls start at full clock.
- PSUM chunk groups of 2-4 banks (7 banks in the pool + 1 warmup) keep
  bank recycling ahead of the drains.
- Output chunks drain (with scale) into per-(img,oc-block) SBUF tiles,
  stored as half-images (small per-group stores for the last image to
  shorten the kernel tail); stores ride the SP ring, image loads the
  ACT/DVE rings.
"""

import numpy as np

N_CORES = 8
IMGS = 4  # images per core
IC = 256
OC = 256
H = W = 56
SEG = 57  # 57-col segment: col0 = left pad (shared as prev seg's right pad)
RSTR = 2 * SEG  # padded row stride: slot0 seg + slot1 seg
NPROWS = 58  # 56 data rows + top/bottom pad
IMG_DATA = NPROWS * RSTR  # 6612
IMG_F = 6624  # padded to mult of 16; tail slack is zeroed (window overrun)
ROWS = 8  # output rows per PSUM tile
NFREE = ROWS * SEG  # 456 <= 512 (PSUM bank limit)
NCHUNK = H // ROWS  # 7
G_ROWS = 8  # image-0 streaming row-group size
N_G = H // G_ROWS  # 7
STEADY_SPLIT = 36  # steady images: ACT signs rows [0,36), DVE [36,56)

_cache = {}

# Skip the Tile kernel-tail drain + semaphore-clear protocol entirely.  The
# trace shows it costs ~8us after the last store (a per-engine semaphore
# handshake storm).  Safe iff the runtime hands each execution zeroed
# semaphores (verified empirically: repeated executions stay bit-exact).
SKIP_TAIL = True


def _install_drain_patch():
    """This walrus build rejects >1 sync-wait on ctrl-type instructions;
    Tile's kernel-tail drain carries one wait per pending proc.  Split it
    into one drain per proc (each with <=1 wait)."""
    import concourse.tile as _tile
    from concourse.vector_clock import ScopedClock, VectorClock

    if getattr(_tile.TileContext, "_drain_split_patch", False):
        return

    def _drain_and_barrier(self, tick_clock, wait_clock):
        nc = self.nc
        assert self.sems is not None
        popped = nc._tile_sem_poison_stack.pop()
        assert popped is self._sem_poison
        if SKIP_TAIL:
            # No drains, no sem clear: the engine streams simply end.  NRT
            # completion waits for all DMA queues regardless, and each
            # execution starts with freshly-zeroed semaphores.
            return
        # Drains AND the sem clears both run on the Pool engine (the
        # framework's clear_and_free_semaphores emits gpsimd dma_reset/
        # sem_clear), so the first all-engine barrier of the stock tail is
        # unnecessary: once Pool's drains observe every proc's final tick,
        # all sem increments have retired and Pool may clear immediately.
        gclock = tick_clock.global_clock
        n = len(gclock)
        for p in range(n):
            t = gclock[p]
            if t <= 0:
                continue
            vec = [0] * n
            vec[p] = t
            d = nc.gpsimd.drain()
            wait_clock.add_sem_waits(d.ins, ScopedClock({None: VectorClock(vec)}))
        nc.clear_and_free_semaphores(list(self.sems.allocated().values()))
        # No final all-engine barrier: every other engine's stream simply
        # ends, NRT completion waits for all queues anyway, and the next
        # invocation cannot start before this one fully retires.

    _tile.TileContext._drain_and_barrier = _drain_and_barrier
    _tile.TileContext._drain_split_patch = True


def _split_excess_waits(nc, maxw=1):
    """Same walrus limitation: hoist excess sync-waits onto same-engine
    NoOps inserted just before the instruction (engine streams are
    in-order, so a preceding NoOp carrying the waits is equivalent)."""
    import concourse.mybir as mybir

    n_split = 0
    for f in nc.m.functions:
        for bb in f.blocks:
            out = []
            for ins in bb.instructions:
                si = ins.sync_info
                if si and si.on_wait and len(si.on_wait) > maxw:
                    waits = list(si.on_wait)
                    excess, keep = waits[:-maxw], waits[-maxw:]
                    for i in range(0, len(excess), maxw):
                        nop = mybir.InstNoOp(
                            name=f"{ins.name}_waitsplit{i}",
                            engine=ins.engine,
                            ins=[],
                            outs=[],
                            sync_info=mybir.SyncInfo(
                                on_wait=excess[i : i + maxw], on_update=[]
                            ),
                        )
                        out.append(nop)
                    si.on_wait = keep
                    n_split += 1
                out.append(ins)
            bb.instructions = out
    return n_split


def build_nc():
    import concourse.bass as bass
    import concourse.mybir as mybir
    from concourse.tile import TileContext

    _install_drain_patch()

    f32 = mybir.dt.float32
    bdt = mybir.dt.float8e4
    Copy = mybir.ActivationFunctionType.Copy
    DR = mybir.MatmulPerfMode.DoubleRow

    nc = bass.Bass()
    x = nc.declare_dram_parameter("x", [IMGS, IC, H, W], f32, isOutput=False)
    # host-binarized weights: [p=128(ic in block), 9 taps x 2 icb, 256 oc] fp8
    wtb = nc.declare_dram_parameter("wtb", [128, 18, OC], bdt, isOutput=False)
    scale = nc.declare_dram_parameter("scale", [OC], f32, isOutput=False)
    y = nc.declare_dram_parameter("y", [IMGS, OC, H, W], f32, isOutput=True)

    with TileContext(nc) as tc:
        with (
            tc.tile_pool(name="const", bufs=1) as cpool,
            tc.tile_pool(name="g0", bufs=7) as g_pool,
            tc.tile_pool(name="tmp", bufs=4) as tmp_pool,
            tc.tile_pool(name="xin", bufs=2) as xin_pool,
            tc.tile_pool(name="outp", bufs=4) as out_pool,
            tc.tile_pool(name="psum", bufs=7, space="PSUM") as psum_pool,
            tc.tile_pool(name="warm", bufs=1, space="PSUM") as warm_pool,
        ):
            # --- PE p-state warmup: tiny self-contained matmuls with no
            # data deps keep the PE array busy (ramping DVFS) while the
            # first image DMAs land; sized to finish just before the first
            # real matmul's inputs are ready.  Inputs are a zeroed scratch
            # tile; the PSUM bank is dedicated and never read.
            warmw = cpool.tile([128, 2, 128], bdt)
            nc.vector.memset(warmw[:], 0.0)
            warmp = warm_pool.tile([128, 64], f32)
            for _ in range(46):
                nc.tensor.matmul(
                    warmp[:], warmw[:], warmw[:, :, 0:64],
                    start=True, stop=True, perf_mode=DR, skip_group_check=True,
                )

            # --- weights (pre-signed fp8 from host) on the ACT ring, which
            # carries nothing else at startup: taps 0-1 first (they gate the
            # first matmuls), then the rest.
            wb = cpool.tile([128, 18, OC], bdt)
            nc.scalar.dma_start(out=wb[:, 0:4, :], in_=wtb[:, 0:4, :])
            nc.scalar.dma_start(out=wb[:, 4:18, :], in_=wtb[:, 4:18, :])

            # --- padding rings: data-independent, zeroed on the GpSimd
            # engine.  Image 0's pads first (they gate its signs), the rest
            # after image 0's streaming loads are on the SWDGE ring.
            xp = cpool.tile([128, IMGS, IMG_F], bdt)

            def pad_image(n):
                v = xp[:, n, 0:IMG_DATA].rearrange("p (r s c) -> p r s c", s=2, c=SEG)
                nc.gpsimd.memset(xp[:, n, 0:RSTR], 0.0)  # top pad row
                nc.gpsimd.memset(xp[:, n, (NPROWS - 1) * RSTR : IMG_F], 0.0)  # bottom+tail
                nc.gpsimd.memset(v[:, 1 : NPROWS - 1, :, 0:1], 0.0)  # left pads

            pad_image(0)

            # scale first on the SP queue (tiny; the first PSUM drains wait
            # on it, so it must not sit behind image loads)
            sc = cpool.tile([128, 2], f32)
            nc.sync.dma_start(out=sc[:], in_=scale.rearrange("(b p) -> p b", p=128))

            def img_view(n):
                return xp[:, n, 0:IMG_DATA].rearrange(
                    "p (r s c) -> p r s c", s=2, c=SEG
                )

            def sign_dst(n, icb, r0, r1):
                # x rows [r0, r1) of slot icb -> padded rows [r0+1, r1+1),
                # data cols 1..56 of that slot's segment
                return img_view(n)[:, r0 + 1 : r1 + 1, icb, 1:57]

            def act_sign(n, icb, xin_slice, r0, r1):
                nc.scalar.sign(sign_dst(n, icb, r0, r1), xin_slice)

            def dve_sign(n, icb, xin_slice, r0, r1, tag="tmp"):
                # sign() on the vector engine: clamp(x * 1e38, -1, 1).
                # Exact for fp32 normals and +-0; frees ScalarE.
                tmp = tmp_pool.tile(
                    [128, r1 - r0, W], f32, name=f"tmp{n}_{icb}_{r0}", tag=tag
                )
                nc.vector.tensor_scalar(
                    tmp[:], xin_slice, 1.0e38, -1.0,
                    op0=mybir.AluOpType.mult, op1=mybir.AluOpType.max,
                )
                nc.vector.tensor_scalar_min(sign_dst(n, icb, r0, r1), tmp[:], 1.0)

            # Both ic blocks of an image (or row range) load in ONE DMA:
            # the DRAM AP folds channel = icb*128 + p into [p, icb, h, w].
            xpair = x.rearrange("n (b p) h w -> n p b h w", p=128)

            def load_pair(n, r0, r1, eng, name):
                t = g_pool.tile(
                    [128, 2, r1 - r0, W], f32, name=name, tag="g0in"
                ) if r1 - r0 <= 8 else xin_pool.tile(
                    [128, 2, H, W], f32, name=name, tag="xin"
                )
                if r1 - r0 <= 8:
                    eng.dma_start(out=t[:], in_=xpair[n, :, :, r0:r1, :])
                else:
                    # steady image: two half-DMAs on different queues so
                    # both halves stream in parallel; the split matches the
                    # ACT/DVE sign split so each engine's sign starts as
                    # soon as its half lands.
                    s = STEADY_SPLIT
                    nc.sync.dma_start(
                        out=t[:, :, 0:s, :], in_=xpair[n, :, :, 0:s, :]
                    )
                    nc.gpsimd.dma_start(
                        out=t[:, :, s:H, :], in_=xpair[n, :, :, s:H, :]
                    )
                return t

            # --- image 0: stream 8-row groups, one pair-DMA each,
            # alternating the SP and GpSimd-SWDGE queues (DMA issues
            # serialize at ~0.6us on a sequencer).  Signs: even groups on
            # ACT (one op per slot), odd on DVE; group boxes stay
            # engine-disjoint.  Chunk c's matmuls unlock once rows <=
            # 8c+9 are signed (range-based deps).
            g_tiles = []
            for g in range(N_G):
                r0, r1 = g * G_ROWS, (g + 1) * G_ROWS
                eng = nc.sync if g % 2 == 0 else nc.gpsimd
                g_tiles.append(load_pair(0, r0, r1, eng, f"g{g}"))
            for g in range(N_G):
                r0, r1 = g * G_ROWS, (g + 1) * G_ROWS
                t = g_tiles[g]
                if g % 2 == 0:
                    act_sign(0, 0, t[:, 0, :, :], r0, r1)
                    act_sign(0, 1, t[:, 1, :, :], r0, r1)
                else:
                    dve_sign(0, 0, t[:, 0, :, :], r0, r1)
                    dve_sign(0, 1, t[:, 1, :, :], r0, r1)

            # remaining pads behind image 0's streaming loads
            for n in range(1, IMGS):
                pad_image(n)

            def load_image(n, eng):
                return load_pair(n, 0, H, eng, f"xin{n}")

            def sign_image(n, xin):
                # ACT signs rows [0, SPLIT) of both slots, DVE the rest:
                # row-disjoint boxes so the engines run in parallel.
                s = STEADY_SPLIT
                act_sign(n, 0, xin[:, 0, 0:s, :], 0, s)
                act_sign(n, 1, xin[:, 1, 0:s, :], 0, s)
                dve_sign(n, 0, xin[:, 0, s:H, :], s, H)
                dve_sign(n, 1, xin[:, 1, s:H, :], s, H)

            def store(n, ocb, c0, c1, eng):
                eng.dma_start(
                    out=y[n, ocb * 128 : (ocb + 1) * 128, c0 * ROWS : c1 * ROWS, :],
                    in_=_obufs[(n, ocb)][:, c0 * ROWS * W : c1 * ROWS * W].rearrange(
                        "p (h w) -> p h w", w=W
                    ),
                )

            _obufs = {}

            def compute_image(n, subs, small_stores=False, mid=None):
                # tap-outer (weight-stationary) within each chunk subgroup so
                # consecutive matmuls hit different PSUM banks; subgroups of
                # 2-4 banks (pool holds 7) keep allocation ahead of drains.
                # `mid` (next image's signs) is emitted after the first
                # subgroup so it lands mid-stream on ACT/DVE: after this
                # image's early drains, before its late ones.
                for ocb in range(2):
                    _obufs[(n, ocb)] = out_pool.tile(
                        [128, H * W], f32, name=f"ob{n}{ocb}", tag="ob"
                    )
                for sub_i, (c0, c1) in enumerate(subs):
                    if sub_i == 1 and mid is not None:
                        mid()
                    for ocb in range(2):
                        psums = [
                            psum_pool.tile(
                                [128, NFREE], f32, name=f"ps{n}{ocb}{c}", tag="ps"
                            )
                            for c in range(c0, c1)
                        ]
                        for t in range(9):
                            kh, kw = divmod(t, 3)
                            lhsT = wb[:, 2 * t : 2 * t + 2, ocb * 128 : (ocb + 1) * 128]
                            for c in range(c0, c1):
                                off = (ROWS * c + kh) * RSTR + kw
                                rhs = xp[:, n, off : off + ROWS * RSTR].rearrange(
                                    "p (r s c) -> p s r c", s=2, c=SEG
                                )
                                nc.tensor.matmul(
                                    psums[c - c0][:],
                                    lhsT,
                                    rhs,
                                    start=(t == 0),
                                    stop=(t == 8),
                                    perf_mode=DR,
                                )
                        for c in range(c0, c1):
                            src = psums[c - c0].rearrange("p (h w) -> p h w", w=SEG)[
                                :, :, 0:W
                            ]
                            dst = _obufs[(n, ocb)][
                                :, c * ROWS * W : (c + 1) * ROWS * W
                            ].rearrange("p (h w) -> p h w", w=W)
                            # drains: DVE takes 5/7 chunks, ACT 2/7 (ACT also
                            # carries weight/store DMA issues); chunks 5 and
                            # 6 are on different engines so the final image's
                            # last two drains run in parallel
                            if c in (1, 5):
                                nc.scalar.activation(
                                    dst, src, Copy, scale=sc[:, ocb : ocb + 1]
                                )
                            else:
                                nc.vector.tensor_scalar_mul(
                                    dst, src, sc[:, ocb : ocb + 1]
                                )
                            if small_stores:
                                # last image: store every chunk as soon as
                                # it drains — the kernel-end barrier waits
                                # for the slowest DMA queue, so spread the
                                # early chunks and put the final ones on
                                # the queues with nothing else pending.
                                if c < 4:
                                    eng = (nc.sync, nc.scalar, nc.gpsimd)[
                                        (c * 2 + ocb) % 3
                                    ]
                                else:
                                    eng = (nc.gpsimd, nc.scalar)[ocb]
                                store(n, ocb, c, c + 1, eng)
                if not small_stores:
                    # halves: oc-block 0 on the (store-only) ACT queue,
                    # oc-block 1 split over SP/GpSimd against their loads
                    store(n, 0, 0, 4, nc.scalar)
                    store(n, 0, 4, NCHUNK, nc.scalar)
                    store(n, 1, 0, 4, nc.gpsimd)
                    store(n, 1, 4, NCHUNK, nc.sync)

            # interleave: image n+1's loads are emitted (and thus queue-
            # prioritized) ahead of image n's compute; its signs are emitted
            # MID-compute so they slot between image n's early and late
            # drains in the ACT/DVE streams (a sign emitted before the
            # drains would block them in-order while waiting for its load).
            xin1 = load_image(1, nc.sync)
            compute_image(
                0, subs=((0, 2), (2, 4), (4, NCHUNK)),
                mid=lambda: sign_image(1, xin1),
            )
            xin2 = load_image(2, nc.gpsimd)
            compute_image(
                1, subs=((0, 4), (4, NCHUNK)),
                mid=lambda: sign_image(2, xin2),
            )
            xin3 = load_image(3, nc.sync)
            compute_image(
                2, subs=((0, 4), (4, NCHUNK)),
                mid=lambda: sign_image(3, xin3),
            )
            compute_image(
                3, subs=((0, 3), (3, 5), (5, 6), (6, NCHUNK)), small_stores=True
            )

    _split_excess_waits(nc)
    return nc


def _get_nc():
    if "nc" not in _cache:
        _cache["nc"] = build_nc()
    return _cache["nc"]


def _pack_weights(weight):
    """Host-side: binarize + lay out conv weights as [p, (kh kw icb), oc] fp8.

    Weight row k of the DoubleRow contraction is icb*128 + p, matching the
    rhs slot order (slot = icb)."""
    import concourse.mybir as mybir

    w = np.sign(weight.astype(np.float32))  # [oc, ic, kh, kw]
    w = w.transpose(2, 3, 1, 0).reshape(3 * 3, 2, 128, OC)  # [t, icb, p, oc]
    w = w.transpose(2, 0, 1, 3).reshape(128, 18, OC)  # [p, (t icb), oc]
    return np.ascontiguousarray(w).astype(mybir.dt.np(mybir.dt.float8e4))


def run(inputs, trace=False, trace_cores=None):
    from concourse.bass_utils import run_bass_kernel_spmd

    x = np.asarray(inputs["x"])
    weight = np.asarray(inputs["weight"])
    scale = np.asarray(inputs["scale"]).astype(np.float32)
    wtb = _pack_weights(weight)

    in_maps = [
        {"x": x[i * IMGS : (i + 1) * IMGS], "wtb": wtb, "scale": scale}
        for i in range(N_CORES)
    ]
    res = run_bass_kernel_spmd(
        _get_nc(),
        in_maps,
        core_ids=list(range(N_CORES)),
        trace=trace,
        trace_cores=trace_cores,
    )
    out = np.concatenate([res.results[i]["y"] for i in range(N_CORES)], axis=0)
    return out, res


def kernel(**inputs):
    # One retry: a previously crashed process can leave a core wedged
    # (NRT_EXEC_UNIT_UNRECOVERABLE); the runtime recovers on the next
    # attempt.
    try:
        out, _ = run(inputs, trace=False)
    except Exception:
        out, _ = run(inputs, trace=False)
    return out


# revision 42
# speedup vs baseline: 1.0326x; 1.0068x over previous
"""Binary conv2d (XNOR-style) + per-channel scale for Trainium2.

y = conv2d(sign(x), sign(w), stride=1, pad=1) * scale[oc]

Data-parallel over batch across 8 NeuronCores (4 images each).

Per core, each image is binarized into a slot-interleaved padded fp8
layout [p=128, row(58), slot(2), 57]: the two in-channel blocks (slots)
of a padded row sit in adjacent 57-col segments, so a DoubleRow matmul
rhs AP [p, 2(slot @57), 8(row @114), 57(col @1)] has a row-local byte
footprint.  That makes the Tile framework's range-based dependencies
row-granular: image 0 streams in 8-row groups (DMA -> sign -> matmul)
instead of waiting for the whole image, and the 57-stride pad-sharing
trick still works because col 0 of every segment is a zeroed left pad.

The 3x3 conv is 9 accumulating DoubleRow matmuls (K=256) per 8-row
output chunk into a PSUM bank; per-channel scale is applied in fp32 on
the PSUM->SBUF drain, so the result is bit-identical to the reference.
Weights are pre-binarized and packed to fp8 on the host (they are
static parameters), quartering their DMA and removing on-chip signs.

Schedule notes (all verified against NTFF traces):
- ~46 tiny dummy matmuls at t=0 ramp the PE DVFS p-state while the
  first DMAs land, so real matmuls start near full clock.
- DMA issues serialize at ~0.6us on their sequencer and each of the 3
  queues (SP, ACT, GpSimd-SWDGE) sustains only ~150-250 GB/s, so both
  issue counts and per-queue byte loads are balanced: one pair-DMA
  loads both ic blocks of a row range ([p, 2, h, w] via a folded DRAM
  AP); weights ride the ACT queue; steady images load as two row
  halves on SP/GpSimd matching the ACT/DVE sign split; stores spread
  over all three queues, per-chunk for the last image (the kernel-end
  barrier waits for the slowest DMA queue).
- A next image's signs are emitted MID-compute of the previous image
  so they sit between its early and late drains in the in-order
  ACT/DVE streams — emitted earlier they would block the drains (and
  thus PSUM bank recycling) while waiting for their loads.
- PSUM chunk groups of 2-4 banks (7 banks in the pool + 1 warmup) keep
  bank recycling ahead of the drains; with tap-outer order inside a
  group, consecutive matmuls never touch the same bank.
- The Tile kernel-tail drain+semaphore-clear protocol (~8us of
  per-engine handshakes) is skipped entirely; each execution starts
  with freshly-zeroed semaphores (verified: repeated runs bit-exact).
"""

import numpy as np

N_CORES = 8
IMGS = 4  # images per core
IC = 256
OC = 256
H = W = 56
SEG = 57  # 57-col segment: col0 = left pad (shared as prev seg's right pad)
RSTR = 2 * SEG  # padded row stride: slot0 seg + slot1 seg
NPROWS = 58  # 56 data rows + top/bottom pad
IMG_DATA = NPROWS * RSTR  # 6612
IMG_F = 6624  # padded to mult of 16; tail slack is zeroed (window overrun)
ROWS = 8  # output rows per PSUM tile
NFREE = ROWS * SEG  # 456 <= 512 (PSUM bank limit)
NCHUNK = H // ROWS  # 7
G_ROWS = 8  # image-0 streaming row-group size
N_G = H // G_ROWS  # 7
STEADY_SPLIT = 36  # steady images: ACT signs rows [0,36), DVE [36,56)

_cache = {}

# Skip the Tile kernel-tail drain + semaphore-clear protocol entirely.  The
# trace shows it costs ~8us after the last store (a per-engine semaphore
# handshake storm).  Safe iff the runtime hands each execution zeroed
# semaphores (verified empirically: repeated executions stay bit-exact).
SKIP_TAIL = True


def _install_drain_patch():
    """This walrus build rejects >1 sync-wait on ctrl-type instructions;
    Tile's kernel-tail drain carries one wait per pending proc.  Split it
    into one drain per proc (each with <=1 wait)."""
    import concourse.tile as _tile
    from concourse.vector_clock import ScopedClock, VectorClock

    if getattr(_tile.TileContext, "_drain_split_patch", False):
        return

    def _drain_and_barrier(self, tick_clock, wait_clock):
        nc = self.nc
        assert self.sems is not None
        popped = nc._tile_sem_poison_stack.pop()
        assert popped is self._sem_poison
        if SKIP_TAIL:
            # No drains, no sem clear: the engine streams simply end.  NRT
            # completion waits for all DMA queues regardless, and each
            # execution starts with freshly-zeroed semaphores.
            return
        # Drains AND the sem clears both run on the Pool engine (the
        # framework's clear_and_free_semaphores emits gpsimd dma_reset/
        # sem_clear), so the first all-engine barrier of the stock tail is
        # unnecessary: once Pool's drains observe every proc's final tick,
        # all sem increments have retired and Pool may clear immediately.
        gclock = tick_clock.global_clock
        n = len(gclock)
        for p in range(n):
            t = gclock[p]
            if t <= 0:
                continue
            vec = [0] * n
            vec[p] = t
            d = nc.gpsimd.drain()
            wait_clock.add_sem_waits(d.ins, ScopedClock({None: VectorClock(vec)}))
        nc.clear_and_free_semaphores(list(self.sems.allocated().values()))
        # No final all-engine barrier: every other engine's stream simply
        # ends, NRT completion waits for all queues anyway, and the next
        # invocation cannot start before this one fully retires.

    _tile.TileContext._drain_and_barrier = _drain_and_barrier
    _tile.TileContext._drain_split_patch = True


def _split_excess_waits(nc, maxw=1):
    """Same walrus limitation: hoist excess sync-waits onto same-engine
    NoOps inserted just before the instruction (engine streams are
    in-order, so a preceding NoOp carrying the waits is equivalent)."""
    import concourse.mybir as mybir

    n_split = 0
    for f in nc.m.functions:
        for bb in f.blocks:
            out = []
            for ins in bb.instructions:
                si = ins.sync_info
                if si and si.on_wait and len(si.on_wait) > maxw:
                    waits = list(si.on_wait)
                    excess, keep = waits[:-maxw], waits[-maxw:]
                    for i in range(0, len(excess), maxw):
                        nop = mybir.InstNoOp(
                            name=f"{ins.name}_waitsplit{i}",
                            engine=ins.engine,
                            ins=[],
                            outs=[],
                            sync_info=mybir.SyncInfo(
                                on_wait=excess[i : i + maxw], on_update=[]
                            ),
                        )
                        out.append(nop)
                    si.on_wait = keep
                    n_split += 1
                out.append(ins)
            bb.instructions = out
    return n_split


def build_nc():
    import concourse.bass as bass
    import concourse.mybir as mybir
    from concourse.tile import TileContext

    _install_drain_patch()

    f32 = mybir.dt.float32
    bdt = mybir.dt.float8e4
    Copy = mybir.ActivationFunctionType.Copy
    DR = mybir.MatmulPerfMode.DoubleRow

    nc = bass.Bass()
    x = nc.declare_dram_parameter("x", [IMGS, IC, H, W], f32, isOutput=False)
    # host-binarized weights: [p=128(ic in block), 9 taps x 2 icb, 256 oc] fp8
    wtb = nc.declare_dram_parameter("wtb", [128, 18, OC], bdt, isOutput=False)
    scale = nc.declare_dram_parameter("scale", [OC], f32, isOutput=False)
    y = nc.declare_dram_parameter("y", [IMGS, OC, H, W], f32, isOutput=True)

    with TileContext(nc) as tc:
        with (
            tc.tile_pool(name="const", bufs=1) as cpool,
            tc.tile_pool(name="g0", bufs=7) as g_pool,
            tc.tile_pool(name="tmp", bufs=4) as tmp_pool,
            tc.tile_pool(name="xin", bufs=2) as xin_pool,
            tc.tile_pool(name="outp", bufs=4) as out_pool,
            tc.tile_pool(name="psum", bufs=7, space="PSUM") as psum_pool,
            tc.tile_pool(name="warm", bufs=1, space="PSUM") as warm_pool,
        ):
            # --- PE p-state warmup: tiny self-contained matmuls with no
            # data deps keep the PE array busy (ramping DVFS) while the
            # first image DMAs land; sized to finish just before the first
            # real matmul's inputs are ready.  Inputs are a zeroed scratch
            # tile; the PSUM bank is dedicated and never read.
            warmw = cpool.tile([128, 2, 128], bdt)
            nc.vector.memset(warmw[:], 0.0)
            warmp = warm_pool.tile([128, 64], f32)
            for _ in range(46):
                nc.tensor.matmul(
                    warmp[:], warmw[:], warmw[:, :, 0:64],
                    start=True, stop=True, perf_mode=DR, skip_group_check=True,
                )

            # --- weights (pre-signed fp8 from host) on the ACT ring, which
            # carries nothing else at startup: taps 0-1 first (they gate the
            # first matmuls), then the rest.
            wb = cpool.tile([128, 18, OC], bdt)
            nc.scalar.dma_start(out=wb[:, 0:4, :], in_=wtb[:, 0:4, :])
            nc.scalar.dma_start(out=wb[:, 4:18, :], in_=wtb[:, 4:18, :])

            # --- padding rings: data-independent, zeroed on the GpSimd
            # engine.  Image 0's pads first (they gate its signs), the rest
            # after image 0's streaming loads are on the SWDGE ring.
            xp = cpool.tile([128, IMGS, IMG_F], bdt)

            def pad_image(n):
                v = xp[:, n, 0:IMG_DATA].rearrange("p (r s c) -> p r s c", s=2, c=SEG)
                nc.gpsimd.memset(xp[:, n, 0:RSTR], 0.0)  # top pad row
                nc.gpsimd.memset(xp[:, n, (NPROWS - 1) * RSTR : IMG_F], 0.0)  # bottom+tail
                nc.gpsimd.memset(v[:, 1 : NPROWS - 1, :, 0:1], 0.0)  # left pads

            pad_image(0)

            # scale first on the SP queue (tiny; the first PSUM drains wait
            # on it, so it must not sit behind image loads)
            sc = cpool.tile([128, 2], f32)
            nc.sync.dma_start(out=sc[:], in_=scale.rearrange("(b p) -> p b", p=128))

            def img_view(n):
                return xp[:, n, 0:IMG_DATA].rearrange(
                    "p (r s c) -> p r s c", s=2, c=SEG
                )

            def sign_dst(n, icb, r0, r1):
                # x rows [r0, r1) of slot icb -> padded rows [r0+1, r1+1),
                # data cols 1..56 of that slot's segment
                return img_view(n)[:, r0 + 1 : r1 + 1, icb, 1:57]

            def act_sign(n, icb, xin_slice, r0, r1):
                nc.scalar.sign(sign_dst(n, icb, r0, r1), xin_slice)

            def dve_sign(n, icb, xin_slice, r0, r1, tag="tmp"):
                # sign() on the vector engine: clamp(x * 1e38, -1, 1).
                # Exact for fp32 normals and +-0; frees ScalarE.
                tmp = tmp_pool.tile(
                    [128, r1 - r0, W], f32, name=f"tmp{n}_{icb}_{r0}", tag=tag
                )
                nc.vector.tensor_scalar(
                    tmp[:], xin_slice, 1.0e38, -1.0,
                    op0=mybir.AluOpType.mult, op1=mybir.AluOpType.max,
                )
                nc.vector.tensor_scalar_min(sign_dst(n, icb, r0, r1), tmp[:], 1.0)

            # Both ic blocks of an image (or row range) load in ONE DMA:
            # the DRAM AP folds channel = icb*128 + p into [p, icb, h, w].
            xpair = x.rearrange("n (b p) h w -> n p b h w", p=128)

            def load_pair(n, r0, r1, eng, name):
                t = g_pool.tile(
                    [128, 2, r1 - r0, W], f32, name=name, tag="g0in"
                ) if r1 - r0 <= 8 else xin_pool.tile(
                    [128, 2, H, W], f32, name=name, tag="xin"
                )
                if r1 - r0 <= 8:
                    eng.dma_start(out=t[:], in_=xpair[n, :, :, r0:r1, :])
                else:
                    # steady image: two half-DMAs on different queues so
                    # both halves stream in parallel; the split matches the
                    # ACT/DVE sign split so each engine's sign starts as
                    # soon as its half lands.
                    s = STEADY_SPLIT
                    nc.sync.dma_start(
                        out=t[:, :, 0:s, :], in_=xpair[n, :, :, 0:s, :]
                    )
                    nc.gpsimd.dma_start(
                        out=t[:, :, s:H, :], in_=xpair[n, :, :, s:H, :]
                    )
                return t

            # --- image 0: stream 8-row groups, one pair-DMA each,
            # alternating the SP and GpSimd-SWDGE queues (DMA issues
            # serialize at ~0.6us on a sequencer).  Signs: even groups on
            # ACT (one op per slot), odd on DVE; group boxes stay
            # engine-disjoint.  Chunk c's matmuls unlock once rows <=
            # 8c+9 are signed (range-based deps).
            g_tiles = []
            for g in range(N_G):
                r0, r1 = g * G_ROWS, (g + 1) * G_ROWS
                eng = nc.sync if g % 2 == 0 else nc.gpsimd
                g_tiles.append(load_pair(0, r0, r1, eng, f"g{g}"))
            for g in range(N_G):
                r0, r1 = g * G_ROWS, (g + 1) * G_ROWS
                t = g_tiles[g]
                if g % 2 == 0:
                    act_sign(0, 0, t[:, 0, :, :], r0, r1)
                    act_sign(0, 1, t[:, 1, :, :], r0, r1)
                else:
                    dve_sign(0, 0, t[:, 0, :, :], r0, r1)
                    dve_sign(0, 1, t[:, 1, :, :], r0, r1)

            # remaining pads behind image 0's streaming loads
            for n in range(1, IMGS):
                pad_image(n)

            def load_image(n, eng):
                return load_pair(n, 0, H, eng, f"xin{n}")

            def sign_image(n, xin):
                # ACT signs rows [0, SPLIT) of both slots, DVE the rest:
                # row-disjoint boxes so the engines run in parallel.
                s = STEADY_SPLIT
                act_sign(n, 0, xin[:, 0, 0:s, :], 0, s)
                act_sign(n, 1, xin[:, 1, 0:s, :], 0, s)
                dve_sign(n, 0, xin[:, 0, s:H, :], s, H)
                dve_sign(n, 1, xin[:, 1, s:H, :], s, H)

            def store(n, ocb, c0, c1, eng):
                eng.dma_start(
                    out=y[n, ocb * 128 : (ocb + 1) * 128, c0 * ROWS : c1 * ROWS, :],
                    in_=_obufs[(n, ocb)][:, c0 * ROWS * W : c1 * ROWS * W].rearrange(
                        "p (h w) -> p h w", w=W
                    ),
                )

            _obufs = {}

            def compute_image(n, subs, small_stores=False, mid=None):
                # tap-outer (weight-stationary) within each chunk subgroup so
                # consecutive matmuls hit different PSUM banks; subgroups of
                # 2-4 banks (pool holds 7) keep allocation ahead of drains.
                # `mid` (next image's signs) is emitted after the first
                # subgroup so it lands mid-stream on ACT/DVE: after this
                # image's early drains, before its late ones.
                for ocb in range(2):
                    _obufs[(n, ocb)] = out_pool.tile(
                        [128, H * W], f32, name=f"ob{n}{ocb}", tag="ob"
                    )
                for sub_i, (c0, c1) in enumerate(subs):
                    if sub_i == 1 and mid is not None:
                        mid()
                    for ocb in range(2):
                        psums = [
                            psum_pool.tile(
                                [128, NFREE], f32, name=f"ps{n}{ocb}{c}", tag="ps"
                            )
                            for c in range(c0, c1)
                        ]
                        for t in range(9):
                            kh, kw = divmod(t, 3)
                            lhsT = wb[:, 2 * t : 2 * t + 2, ocb * 128 : (ocb + 1) * 128]
                            for c in range(c0, c1):
                                off = (ROWS * c + kh) * RSTR + kw
                                rhs = xp[:, n, off : off + ROWS * RSTR].rearrange(
                                    "p (r s c) -> p s r c", s=2, c=SEG
                                )
                                nc.tensor.matmul(
                                    psums[c - c0][:],
                                    lhsT,
                                    rhs,
                                    start=(t == 0),
                                    stop=(t == 8),
                                    perf_mode=DR,
                                )
                        for c in range(c0, c1):
                            src = psums[c - c0].rearrange("p (h w) -> p h w", w=SEG)[
                                :, :, 0:W
                            ]
                            dst = _obufs[(n, ocb)][
                                :, c * ROWS * W : (c + 1) * ROWS * W
                            ].rearrange("p (h w) -> p h w", w=W)
                            # drains: DVE takes 5/7 chunks, ACT 2/7 (ACT also
                            # carries weight/store DMA issues); chunks 5 and
                            # 6 are on different engines so the final image's
                            # last two drains run in parallel
                            if c in (1, 5):
                                nc.scalar.activation(
                                    dst, src, Copy, scale=sc[:, ocb : ocb + 1]
                                )
                            else:
                                nc.vector.tensor_scalar_mul(
                                    dst, src, sc[:, ocb : ocb + 1]
                                )
                            if small_stores:
                                # last image: store every chunk as soon as
                                # it drains — the kernel-end barrier waits
                                # for the slowest DMA queue, so spread the
                                # early chunks and put the final ones on
                                # the queues with nothing else pending.
                                if c < 4:
                                    eng = (nc.sync, nc.scalar, nc.gpsimd)[
                                        (c * 2 + ocb) % 3
                                    ]
                                else:
                                    eng = (nc.gpsimd, nc.scalar)[ocb]
                                store(n, ocb, c, c + 1, eng)
                if not small_stores:
                    # halves: oc-block 0 on the (store-only) ACT queue,
                    # oc-block 1 split over SP/GpSimd against their loads
                    store(n, 0, 0, 4, nc.scalar)
                    store(n, 0, 4, NCHUNK, nc.scalar)
                    store(n, 1, 0, 4, nc.gpsimd)
                    store(n, 1, 4, NCHUNK, nc.sync)

            # interleave: image n+1's loads are emitted (and thus queue-
            # prioritized) ahead of image n's compute; its signs are emitted
            # MID-compute so they slot between image n's early and late
            # drains in the ACT/DVE streams (a sign emitted before the
            # drains would block them in-order while waiting for its load).
            xin1 = load_image(1, nc.sync)
            compute_image(
                0, subs=((0, 2), (2, 4), (4, NCHUNK)),
                mid=lambda: sign_image(1, xin1),
            )
            xin2 = load_image(2, nc.gpsimd)
            compute_image(
                1, subs=((0, 4), (4, NCHUNK)),
                mid=lambda: sign_image(2, xin2),
            )
            xin3 = load_image(3, nc.sync)
            compute_image(
                2, subs=((0, 4), (4, NCHUNK)),
                mid=lambda: sign_image(3, xin3),
            )
            compute_image(
                3, subs=((0, 3), (3, 5), (5, 6), (6, NCHUNK)), small_stores=True
            )

    _split_excess_waits(nc)
    return nc


def _get_nc():
    if "nc" not in _cache:
        _cache["nc"] = build_nc()
    return _cache["nc"]


def _pack_weights(weight):
    """Host-side: binarize + lay out conv weights as [p, (kh kw icb), oc] fp8.

    Weight row k of the DoubleRow contraction is icb*128 + p, matching the
    rhs slot order (slot = icb)."""
    import concourse.mybir as mybir

    w = np.sign(weight.astype(np.float32))  # [oc, ic, kh, kw]
    w = w.transpose(2, 3, 1, 0).reshape(3 * 3, 2, 128, OC)  # [t, icb, p, oc]
    w = w.transpose(2, 0, 1, 3).reshape(128, 18, OC)  # [p, (t icb), oc]
    return np.ascontiguousarray(w).astype(mybir.dt.np(mybir.dt.float8e4))


def run(inputs, trace=False, trace_cores=None):
    from concourse.bass_utils import run_bass_kernel_spmd

    x = np.asarray(inputs["x"])
    weight = np.asarray(inputs["weight"])
    scale = np.asarray(inputs["scale"]).astype(np.float32)
    wtb = _pack_weights(weight)

    in_maps = [
        {"x": x[i * IMGS : (i + 1) * IMGS], "wtb": wtb, "scale": scale}
        for i in range(N_CORES)
    ]
    res = run_bass_kernel_spmd(
        _get_nc(),
        in_maps,
        core_ids=list(range(N_CORES)),
        trace=trace,
        trace_cores=trace_cores,
    )
    out = np.concatenate([res.results[i]["y"] for i in range(N_CORES)], axis=0)
    return out, res


def kernel(**inputs):
    # One retry: a previously crashed process can leave a core wedged
    # (NRT_EXEC_UNIT_UNRECOVERABLE); the runtime recovers on the next
    # attempt.
    try:
        out, _ = run(inputs, trace=False)
    except Exception:
        out, _ = run(inputs, trace=False)
    return out
